# revision 8
# baseline (speedup 1.0000x reference)
# Self-contained Trainium2 Bass kernel for nn_MultiInputLSTMCell.
#
# Reference computation (all fp32):
#   pre   = h0 @ W_hh + bias + input_ @ W_ih          # (1, 3H)
#   i, o  = sigmoid(pre[:, :H]), sigmoid(pre[:, H:2H])
#   g     = tanh(pre[:, 2H:])
#   awi   = input_ @ aW_ih + a_bias                   # (1, H)
#   awh   = c_input @ aW_hh                           # (C, H)
#   alpha = sigmoid(awi + awh)                        # (C, H)
#   w     = exp([i; alpha]); w /= w.sum(0)            # (C+1, H)
#   c1    = (([g; c_input]) * w).sum(0)               # (1, H)
#   h1    = o * tanh(c1)
#
# Strategy: tensor-parallel over the hidden (output-column) dim across 8
# cores (HS = 256 columns each); everything after the matmuls is local to a
# shard, so no collectives.
#
# v2 design (from trace analysis of the 43.7us bf16 baseline):
#  * Weights are host-quantized to fp8 E3M4 (4 mantissa bits) with a x64
#    pre-scale (x128 for the g block, folding tanh(x)=t(x) directly); the
#    consuming ACT ops descale via their input-scale argument.  Measured
#    end-to-end absmax-rel err ~9e-3, comfortably under the 2e-2 gate, vs
#    2.4e-2 for E4M3 (too coarse).  This halves HBM traffic vs bf16 to
#    ~4.5 MB/core and keeps the PE ingress at 1 col/cycle (float8e3 is
#    full-rate for the moving operand; only e4/e5 double-pump, and those
#    fail precision).
#  * All sigmoids become 0.5 + 0.5*tanh(x/2) and g is a direct tanh: tanh
#    and exp live in the same ACT table set ("exp_and_others"), so the
#    whole kernel runs off one table load with ~half the tail ops of the
#    exp+reciprocal formulation.
#  * Gate columns are split [i|g] (streamed first) and [o] (streamed last):
#    the long serial chain (softmax-normalize -> c1 -> tanh(c1), plus the
#    c1 store) overlaps the o-gate stream+matmuls, leaving only
#    tanh->affine->mul->store for h1 after the last weight byte.
#  * The (C+1)-row exp-normalize reduction runs as a K=64 ones-matmul over
#    the alpha rows mid-stream; the i/g row joins via one DVE add on a
#    packed [s0|s1] PSUM tile (no PE round-trip on the tail).
#  * The PE clock ramps 0.65->1.2->2.4 GHz with ~3us to max: a few dummy
#    matmuls on a memset tile start right after engine init, so the ramp
#    happens before the first weight chunk lands instead of during it.
#  * Single sync-ring DMA stream in exact consumption order (the scalar
#    ring is ~3x slower for bulk); ~375 GB/s effective once flowing.
#
# Measured phases worth knowing: ~7.2us framework preamble + ~2.5us
# teardown are fixed (a 1-DMA kernel measures 13.5us), so the win is all
# in stream/PE/tail time.

import numpy as np

import concourse.bass as bass
import concourse.tile as tile
from concourse import bacc, mybir
from concourse.bass_utils import run_bass_kernel_spmd

NCORES = 8
H = 2048          # hidden size
IN = 2048         # input size
C = 64            # number of skip-word cell states
HS = H // NCORES  # hidden shard per core = 256
KG = IN + H       # gates contraction dim = 4096
KO_G = KG // 128  # 32 k-chunks for gates
KO_A = IN // 128  # 16 k-chunks per alpha matmul
SCALE = 64.0      # fp8 weight pre-scale (g block gets 2*SCALE)
F32 = mybir.dt.float32
F32R = mybir.dt.float32r
BF16 = mybir.dt.bfloat16
FP8 = mybir.dt.float8e3

_nc_cache = None


def _build_nc():
    """Build the single-core Bass program (same program runs on all 8 cores)."""
    nc = bacc.Bacc(
        "TRN2",
        target_bir_lowering=False,
        debug=False,
        enable_asserts=False,
        name="multi_input_lstm_cell",
    )

    # DRAM I/O (per-core shards; shapes identical on every core).
    # Weights are host-pre-tiled to [ki=128, ko, n] so each chunk DMA reads
    # one long contiguous segment per partition.
    # wig columns: [i-gate shard (256) | 2*g-gate shard (256)], fp8 e3m4 * 64
    wig = nc.dram_tensor("wig", [128, KO_G, 2 * HS], FP8, kind="ExternalInput").ap()
    # wo columns: o-gate shard, fp8 e3m4 * 64
    wo = nc.dram_tensor("wo", [128, KO_G, HS], FP8, kind="ExternalInput").ap()
    # walpha rows 0..2047 = alpha_weight_ih shard, 2048..4095 = alpha_weight_hh
    wa = nc.dram_tensor("wa", [128, 2 * KO_A, HS], FP8, kind="ExternalInput").ap()
    # bab = [64*b_i | 128*b_g | 64*b_o | 64*ab]  (f32r so rank-1 matmuls mix
    # with the f32r ones lhsT)
    bab = nc.dram_tensor("bab", [1, 4 * HS], F32R, kind="ExternalInput").ap()
    cs = nc.dram_tensor("cs", [C, HS], F32R, kind="ExternalInput").ap()
    xt = nc.dram_tensor("xt", [128, KO_G], BF16, kind="ExternalInput").ap()
    ct = nc.dram_tensor("ct", [128, KO_A, C], BF16, kind="ExternalInput").ap()
    # hc[0, 0:256] = c1 shard, hc[0, 256:512] = h1 shard
    hc = nc.dram_tensor("hc", [1, 2 * HS], F32, kind="ExternalOutput").ap()

    with tile.TileContext(nc) as tc:
        _emit(tc, wig, wo, wa, bab, cs, xt, ct, hc)

    nc.compile()
    return nc


def _emit(tc, wig, wo, wa, bab, cs, xt, ct, hc):
    from contextlib import ExitStack

    nc = tc.nc
    TANH = mybir.ActivationFunctionType.Tanh
    EXP = mybir.ActivationFunctionType.Exp
    MUL = mybir.AluOpType.mult
    ADD = mybir.AluOpType.add
    INV2S = 1.0 / (2.0 * SCALE)

    with ExitStack() as ctx:
        singles = ctx.enter_context(tc.tile_pool(name="singles", bufs=1))
        ig_pool = ctx.enter_context(tc.tile_pool(name="ig_pool", bufs=4))
        o_pool = ctx.enter_context(tc.tile_pool(name="o_pool", bufs=4))
        psum = ctx.enter_context(tc.tile_pool(name="psum", bufs=1, space="PSUM"))

        # ---- tiles whose data comes from memset (no DMA dependency) -----
        warm_t = singles.tile([128, 512], BF16, tag="warm")
        nc.vector.memset(warm_t[:], 0.0)
        # memset rejects f32r, so set as f32 and bitcast at the matmul sites
        ones_b_f = singles.tile([1, C], F32, tag="ones_b")  # bias/broadcast lhsT
        nc.vector.memset(ones_b_f[:], 1.0)
        ones_b = ones_b_f[:].bitcast(F32R)
        ones_r_f = singles.tile([C, 1], F32, tag="ones_r")  # reduction lhsT
        nc.vector.memset(ones_r_f[:], 1.0)
        ones_r = ones_r_f[:].bitcast(F32R)
        # Pre-warm the ACT exp/tanh table while everything is idle.
        wmt = singles.tile([1, 1], F32, tag="wmt")
        nc.vector.memset(wmt[:], 0.0)
        nc.scalar.activation(out=wmt[:], in_=wmt[:], func=EXP)
        # per-partition 0.5 bias vectors for the exp(0.5*t + 0.5) ops
        half1 = singles.tile([1, 1], F32, tag="half1")
        nc.vector.memset(half1[:], 0.5)
        half64 = singles.tile([C, 1], F32, tag="half64")
        nc.vector.memset(half64[:], 0.5)

        # ---- single big-transfer stream on the sync HWDGE ring, in exact
        # PE consumption order.
        xt_t = singles.tile([128, KO_G], BF16, tag="xt")
        nc.sync.dma_start(out=xt_t[:], in_=xt)
        bab_t = singles.tile([1, 4 * HS], F32R, tag="bab")
        nc.sync.dma_start(out=bab_t[:], in_=bab)

        # ---- PSUM tiles -------------------------------------------------
        pg_ig = psum.tile([1, 2 * HS], F32, tag="pg_ig")  # [pre_i | 2*pre_g]
        pg_o = psum.tile([1, HS], F32, tag="pg_o")        # pre_o
        pwi = psum.tile([1, HS], F32, tag="pwi")          # alpha_wi row
        pal = psum.tile([C, HS], F32, tag="pal")          # alpha pre-activation
        ps = psum.tile([1, 2 * HS], F32, tag="ps")        # [sum_ew | sum_mg]
        pdum = psum.tile([1, 2 * HS], F32, tag="pdum")    # warm scratch

        # ---- PE clock ramp: dummy matmuls on memset data bridge the
        # preamble -> first-weight-chunk window so the 0.65->2.4 GHz p-state
        # ramp completes before real work arrives.
        for _ in range(4):
            nc.tensor.matmul(pdum[:], lhsT=warm_t[:, 0:1], rhs=warm_t[:],
                             start=True, stop=True)

        # gates bias rows via K=1 rank-1 matmuls (open the PSUM groups)
        nc.tensor.matmul(pg_ig[:], lhsT=ones_b[0:1, 0:1], rhs=bab_t[:, 0:512],
                         start=True, stop=False)
        nc.tensor.matmul(pg_o[:], lhsT=ones_b[0:1, 0:1], rhs=bab_t[:, 512:768],
                         start=True, stop=False)

        # ---- [i|g] gates stream: ramp-up chunk sizes so the PE gets work
        # as soon as bytes land; DMA runs ahead of the PE after that.
        ig_sizes = [1, 2, 4, 8, 8, 9]
        k0 = 0
        for sz in ig_sizes:
            t = ig_pool.tile([128, 9, 2 * HS], FP8, tag="wig")
            nc.sync.dma_start(out=t[:, 0:sz, :], in_=wig[:, k0 : k0 + sz, :])
            for km in range(sz):
                kk = k0 + km
                nc.tensor.matmul(
                    pg_ig[:],
                    lhsT=xt_t[:, kk : kk + 1],
                    rhs=t[:, km, :],
                    start=False,
                    stop=(kk == KO_G - 1),
                )
            k0 += sz

        # ---- alpha weights / c_input^T, then o weights ------------------
        wa_t = singles.tile([128, 2 * KO_A, HS], FP8, tag="wa")
        ct_t = singles.tile([128, KO_A, C], BF16, tag="ct")
        nc.sync.dma_start(out=wa_t[:, 0:KO_A, :], in_=wa[:, 0:KO_A, :])
        nc.sync.dma_start(out=ct_t[:], in_=ct)
        nc.sync.dma_start(out=wa_t[:, KO_A : 2 * KO_A, :],
                          in_=wa[:, KO_A : 2 * KO_A, :])
        cs_t = singles.tile([C, HS], F32R, tag="cs")
        nc.sync.dma_start(out=cs_t[:], in_=cs)

        o_sizes = [8, 8, 8, 8]
        o_tiles = []
        k0 = 0
        for sz in o_sizes:
            t = o_pool.tile([128, 8, HS], FP8, tag="wo")
            nc.sync.dma_start(out=t[:, 0:sz, :], in_=wo[:, k0 : k0 + sz, :])
            o_tiles.append((t, k0, sz))
            k0 += sz

        # ---- ig tail part 1 (ACT/DVE; overlaps alpha matmuls below) -----
        # tio = tanh(pre/(2S)) -> [t_i | t_g] with t_g = tanh(pre_g) exactly
        # (g weights are host-doubled).
        tio = singles.tile([1, 2 * HS], F32, tag="tio")
        nc.scalar.activation(out=tio[:], in_=pg_ig[:], func=TANH, scale=INV2S)
        # ewmg = [ew64 | mg64]: ew64 = exp(sig_i) = exp(0.5*t_i + 0.5);
        # mg64 = g * ew64
        ewmg = singles.tile([1, 2 * HS], F32, tag="ewmg")
        nc.scalar.activation(out=ewmg[:, 0:HS], in_=tio[:, 0:HS], func=EXP,
                             scale=0.5, bias=half1[:])
        nc.vector.tensor_tensor(out=ewmg[:, HS : 2 * HS], in0=tio[:, HS : 2 * HS],
                                in1=ewmg[:, 0:HS], op=MUL)

        # ---- alpha matmuls ----------------------------------------------
        # alpha_wi = input_ @ aW_ih  (input_ = xt cols 16..31)
        for ko in range(KO_A):
            nc.tensor.matmul(
                pwi[:],
                lhsT=xt_t[:, KO_A + ko : KO_A + ko + 1],
                rhs=wa_t[:, ko, :],
                start=(ko == 0),
                stop=(ko == KO_A - 1),
            )
        # alpha pre = c_input @ aW_hh
        for ko in range(KO_A):
            nc.tensor.matmul(
                pal[:],
                lhsT=ct_t[:, ko, :],
                rhs=wa_t[:, KO_A + ko, :],
                start=(ko == 0),
                stop=False,
            )
        # wi row (+ alpha_bias) to SBUF, broadcast-added into pal via a K=1
        # rank-1 matmul with a ones column.
        wi_t = singles.tile([1, HS], F32R, tag="wi")
        nc.vector.tensor_tensor(out=wi_t[:], in0=pwi[:], in1=bab_t[:, 768:1024],
                                op=ADD)
        nc.tensor.matmul(pal[:], lhsT=ones_b[0:1, 0:C], rhs=wi_t[:],
                         start=False, stop=True)

        # ---- alpha tail (ACT/DVE; overlaps the o-gate stream) -----------
        tal = singles.tile([C, HS], F32, tag="tal")
        nc.scalar.activation(out=tal[:], in_=pal[:], func=TANH, scale=INV2S)
        ew_al = singles.tile([C, HS], F32R, tag="ew_al")
        nc.scalar.activation(out=ew_al[:], in_=tal[:], func=EXP,
                             scale=0.5, bias=half64[:])
        mg_al = singles.tile([C, HS], F32R, tag="mg_al")
        nc.vector.tensor_tensor(out=mg_al[:], in0=cs_t[:], in1=ew_al[:], op=MUL)

        # ---- o-gate matmuls, reduction interleaved ----------------------
        for oi, (t, k0, sz) in enumerate(o_tiles):
            for km in range(sz):
                kk = k0 + km
                nc.tensor.matmul(
                    pg_o[:],
                    lhsT=xt_t[:, kk : kk + 1],
                    rhs=t[:, km, :],
                    start=False,
                    stop=(kk == KO_G - 1),
                )
            if oi == 1:
                # (C)-axis exp-normalize reductions over the 64 alpha rows
                # (the i/g row joins via a DVE add below, off the PE).
                nc.tensor.matmul(ps[:, 0:HS], lhsT=ones_r[:], rhs=ew_al[:],
                                 start=True, stop=True)
                nc.tensor.matmul(ps[:, HS : 2 * HS], lhsT=ones_r[:], rhs=mg_al[:],
                                 start=True, stop=True)

        # ---- c1 chain (overlaps the tail of the o stream) ---------------
        st = singles.tile([1, 2 * HS], F32, tag="st")
        nc.vector.tensor_tensor(out=st[:], in0=ps[:], in1=ewmg[:], op=ADD)
        r_t = singles.tile([1, HS], F32, tag="r")
        nc.vector.reciprocal_approx_fast(out=r_t[:], in_=st[:, 0:HS])
        hc_t = singles.tile([1, 2 * HS], F32, tag="hc")
        nc.vector.tensor_tensor(out=hc_t[:, 0:HS], in0=st[:, HS : 2 * HS],
                                in1=r_t[:], op=MUL)
        # c1 half goes out immediately, overlapping the rest
        nc.sync.dma_start(out=hc[:, 0:HS], in_=hc_t[:, 0:HS])
        t4 = singles.tile([1, HS], F32, tag="t4")
        nc.scalar.activation(out=t4[:], in_=hc_t[:, 0:HS], func=TANH)

        # ---- o tail: sig_o = 0.5 + 0.5*tanh(pre_o/(2S)); h1 = sig_o*t4 --
        to_t = singles.tile([1, HS], F32, tag="to")
        nc.scalar.activation(out=to_t[:], in_=pg_o[:], func=TANH, scale=INV2S)
        nc.vector.tensor_scalar(out=to_t[:], in0=to_t[:], scalar1=0.5,
                                scalar2=0.5, op0=MUL, op1=ADD)
        nc.vector.tensor_tensor(out=hc_t[:, HS : 2 * HS], in0=to_t[:],
                                in1=t4[:], op=MUL)
        nc.sync.dma_start(out=hc[:, HS : 2 * HS], in_=hc_t[:, HS : 2 * HS])


def _shard_inputs(input_, c_input, h0, c0, weight_ih, weight_hh,
                  alpha_weight_ih, alpha_weight_hh, bias, alpha_bias):
    """Host-side scatter: column-shard the weights over the hidden dim.

    Weights are scaled by 64 (g block by 128), clipped to the e3m4 range and
    cast to fp8 once; per-core shards are then cheap slices.
    """
    import ml_dtypes
    f32 = np.float32
    bf16 = ml_dtypes.bfloat16
    e3m4 = ml_dtypes.float8_e3m4

    x_comb = np.concatenate([h0[0], input_[0]]).astype(f32)          # (4096,)
    xt = np.ascontiguousarray(x_comb.reshape(KO_G, 128).T).astype(bf16)
    # c_input.T tiled to [ki=128, ko=16, C]
    ct = np.ascontiguousarray(
        c_input.T.reshape(KO_A, 128, C)).transpose(1, 0, 2)
    ct = np.ascontiguousarray(ct).astype(bf16)

    def q8(x):
        return np.clip(x * f32(SCALE), -15.5, 15.5).astype(e3m4)

    # gates weights: stack [W_hh; W_ih]; quantize i/o at x64, g at x128.
    wg_full = np.concatenate([weight_hh, weight_ih], axis=0).astype(f32)
    wq_i = q8(wg_full[:, 0:H])
    wq_o = q8(wg_full[:, H : 2 * H])
    wq_g = q8(wg_full[:, 2 * H : 3 * H] * f32(2.0))
    del wg_full

    def tile_k(w):  # [4096, n] -> [128, 32, n]
        n = w.shape[1]
        return np.ascontiguousarray(w.reshape(KO_G, 128, n).transpose(1, 0, 2))

    wa_full = np.concatenate([alpha_weight_ih, alpha_weight_hh], axis=0)
    wa_q = q8(wa_full.astype(f32))
    del wa_full

    bias = np.asarray(bias, f32)
    alpha_bias = np.asarray(alpha_bias, f32)
    c_input = np.asarray(c_input, f32)

    in_maps = []
    for k in range(NCORES):
        cols = np.s_[k * HS : (k + 1) * HS]
        wig = tile_k(np.concatenate([wq_i[:, cols], wq_g[:, cols]], axis=1))
        wo = tile_k(wq_o[:, cols])
        wa = np.ascontiguousarray(
            wa_q[:, cols].reshape(2 * KO_A, 128, HS).transpose(1, 0, 2))
        bab = np.concatenate(
            [bias[0 * H + k * HS : 0 * H + (k + 1) * HS] * f32(SCALE),
             bias[2 * H + k * HS : 2 * H + (k + 1) * HS] * f32(2 * SCALE),
             bias[1 * H + k * HS : 1 * H + (k + 1) * HS] * f32(SCALE),
             alpha_bias[cols] * f32(SCALE)])[None, :].astype(f32)
        in_maps.append({
            "wig": wig,
            "wo": wo,
            "wa": wa,
            "bab": bab,
            "cs": np.ascontiguousarray(c_input[:, cols]),
            "xt": xt,
            "ct": ct,
        })
    return in_maps


def _run(inputs, trace=False):
    global _nc_cache
    if _nc_cache is None:
        _nc_cache = _build_nc()
    nc = _nc_cache
    in_maps = _shard_inputs(**inputs)
    res = run_bass_kernel_spmd(nc, in_maps, core_ids=list(range(NCORES)), trace=trace)
    h1 = np.concatenate(
        [res.results[k]["hc"][:, HS : 2 * HS] for k in range(NCORES)], axis=1)
    c1 = np.concatenate(
        [res.results[k]["hc"][:, 0:HS] for k in range(NCORES)], axis=1)
    return (h1.astype(np.float32), c1.astype(np.float32)), res


def kernel(input_, c_input, h0, c0, weight_ih, weight_hh,
           alpha_weight_ih, alpha_weight_hh, bias, alpha_bias):
    inputs = dict(
        input_=np.asarray(input_, np.float32),
        c_input=np.asarray(c_input, np.float32),
        h0=np.asarray(h0, np.float32),
        c0=np.asarray(c0, np.float32),
        weight_ih=np.asarray(weight_ih, np.float32),
        weight_hh=np.asarray(weight_hh, np.float32),
        alpha_weight_ih=np.asarray(alpha_weight_ih, np.float32),
        alpha_weight_hh=np.asarray(alpha_weight_hh, np.float32),
        bias=np.asarray(bias, np.float32),
        alpha_bias=np.asarray(alpha_bias, np.float32),
    )
    out, _ = _run(inputs)
    return out


# revision 12
# speedup vs baseline: 1.0503x; 1.0503x over previous
# Self-contained Trainium2 Bass kernel for nn_MultiInputLSTMCell.
#
# Reference computation (all fp32):
#   pre   = h0 @ W_hh + bias + input_ @ W_ih          # (1, 3H)
#   i, o  = sigmoid(pre[:, :H]), sigmoid(pre[:, H:2H])
#   g     = tanh(pre[:, 2H:])
#   awi   = input_ @ aW_ih + a_bias                   # (1, H)
#   awh   = c_input @ aW_hh                           # (C, H)
#   alpha = sigmoid(awi + awh)                        # (C, H)
#   w     = exp([i; alpha]); w /= w.sum(0)            # (C+1, H)
#   c1    = (([g; c_input]) * w).sum(0)               # (1, H)
#   h1    = o * tanh(c1)
#
# Strategy: tensor-parallel over the hidden (output-column) dim across 8
# cores (HS = 256 columns each); everything after the matmuls is local to a
# shard, so no collectives.
#
# v2 design (from trace analysis of the 43.7us bf16 baseline):
#  * Weights are host-quantized to fp8 E3M4 (4 mantissa bits) with a x64
#    pre-scale (x128 for the g block, folding tanh(x)=t(x) directly); the
#    consuming ACT ops descale via their input-scale argument.  Measured
#    end-to-end absmax-rel err ~9e-3, comfortably under the 2e-2 gate, vs
#    2.4e-2 for E4M3 (too coarse).  This halves HBM traffic vs bf16 to
#    ~4.5 MB/core and keeps the PE ingress at 1 col/cycle (float8e3 is
#    full-rate for the moving operand; only e4/e5 double-pump, and those
#    fail precision).
#  * All sigmoids become 0.5 + 0.5*tanh(x/2) and g is a direct tanh: tanh
#    and exp live in the same ACT table set ("exp_and_others"), so the
#    whole kernel runs off one table load with ~half the tail ops of the
#    exp+reciprocal formulation.
#  * Gate columns are split [i|g] (streamed first) and [o] (streamed last):
#    the long serial chain (softmax-normalize -> c1 -> tanh(c1), plus the
#    c1 store) overlaps the o-gate stream+matmuls, leaving only
#    tanh->affine->mul->store for h1 after the last weight byte.
#  * The (C+1)-row exp-normalize reduction runs as a K=64 ones-matmul over
#    the alpha rows mid-stream; the i/g row joins via one DVE add on a
#    packed [s0|s1] PSUM tile (no PE round-trip on the tail).
#  * The PE clock ramps 0.65->1.2->2.4 GHz with ~3us to max: a few dummy
#    matmuls on a memset tile start right after engine init, so the ramp
#    happens before the first weight chunk lands instead of during it.
#  * Single sync-ring DMA stream in exact consumption order (the scalar
#    ring is ~3x slower for bulk); ~375 GB/s effective once flowing.
#
# Measured phases worth knowing: ~7.2us framework preamble + ~2.5us
# teardown are fixed (a 1-DMA kernel measures 13.5us), so the win is all
# in stream/PE/tail time.

import numpy as np

import concourse.bass as bass
import concourse.tile as tile
from concourse import bacc, mybir
from concourse.bass_utils import run_bass_kernel_spmd

NCORES = 8
H = 2048          # hidden size
IN = 2048         # input size
C = 64            # number of skip-word cell states
HS = H // NCORES  # hidden shard per core = 256
KG = IN + H       # gates contraction dim = 4096
KO_G = KG // 128  # 32 k-chunks for gates
KO_A = IN // 128  # 16 k-chunks per alpha matmul
SCALE = 64.0      # fp8 weight pre-scale (g block gets 2*SCALE)
F32 = mybir.dt.float32
F32R = mybir.dt.float32r
BF16 = mybir.dt.bfloat16
FP8 = mybir.dt.float8e3

_nc_cache = None


def _build_nc():
    """Build the single-core Bass program (same program runs on all 8 cores)."""
    nc = bacc.Bacc(
        "TRN2",
        target_bir_lowering=False,
        debug=False,
        enable_asserts=False,
        name="multi_input_lstm_cell",
    )

    # DRAM I/O (per-core shards; shapes identical on every core).
    # Weights are host-pre-tiled to [ki=128, ko, n] so each chunk DMA reads
    # one long contiguous segment per partition.
    # wig columns: [i-gate shard (256) | 2*g-gate shard (256)], fp8 e3m4 * 64
    wig = nc.dram_tensor("wig", [128, KO_G, 2 * HS], FP8, kind="ExternalInput").ap()
    # wo columns: o-gate shard, fp8 e3m4 * 64
    wo = nc.dram_tensor("wo", [128, KO_G, HS], FP8, kind="ExternalInput").ap()
    # walpha rows 0..2047 = alpha_weight_ih shard, 2048..4095 = alpha_weight_hh
    wa = nc.dram_tensor("wa", [128, 2 * KO_A, HS], FP8, kind="ExternalInput").ap()
    # bab = [64*b_i | 128*b_g | 64*b_o | 64*ab]  (f32r so rank-1 matmuls mix
    # with the f32r ones lhsT)
    bab = nc.dram_tensor("bab", [1, 4 * HS], F32R, kind="ExternalInput").ap()
    cs = nc.dram_tensor("cs", [C, HS], F32R, kind="ExternalInput").ap()
    xt = nc.dram_tensor("xt", [128, KO_G], BF16, kind="ExternalInput").ap()
    ct = nc.dram_tensor("ct", [128, KO_A, C], BF16, kind="ExternalInput").ap()
    # hc[0, 0:256] = c1 shard, hc[0, 256:512] = h1 shard
    hc = nc.dram_tensor("hc", [1, 2 * HS], F32, kind="ExternalOutput").ap()

    with tile.TileContext(nc) as tc:
        _emit(tc, wig, wo, wa, bab, cs, xt, ct, hc)

    nc.compile()
    return nc


def _emit(tc, wig, wo, wa, bab, cs, xt, ct, hc):
    from contextlib import ExitStack

    nc = tc.nc
    TANH = mybir.ActivationFunctionType.Tanh
    EXP = mybir.ActivationFunctionType.Exp
    MUL = mybir.AluOpType.mult
    ADD = mybir.AluOpType.add
    INV2S = 1.0 / (2.0 * SCALE)

    with ExitStack() as ctx:
        singles = ctx.enter_context(tc.tile_pool(name="singles", bufs=1))
        ig_pool = ctx.enter_context(tc.tile_pool(name="ig_pool", bufs=4))
        o_pool = ctx.enter_context(tc.tile_pool(name="o_pool", bufs=4))
        psum = ctx.enter_context(tc.tile_pool(name="psum", bufs=1, space="PSUM"))

        # ---- tiles whose data comes from memset (no DMA dependency) -----
        warm_t = singles.tile([128, 256], BF16, tag="warm")
        nc.vector.memset(warm_t[:], 0.0)
        # memset rejects f32r, so set as f32 and bitcast at the matmul sites
        ones_b_f = singles.tile([1, C], F32, tag="ones_b")  # bias/broadcast lhsT
        nc.vector.memset(ones_b_f[:], 1.0)
        ones_b = ones_b_f[:].bitcast(F32R)
        ones_r_f = singles.tile([C, 1], F32, tag="ones_r")  # reduction lhsT
        nc.vector.memset(ones_r_f[:], 1.0)
        ones_r = ones_r_f[:].bitcast(F32R)
        # Pre-warm the ACT exp/tanh table while everything is idle.
        wmt = singles.tile([1, 1], F32, tag="wmt")
        nc.vector.memset(wmt[:], 0.0)
        nc.scalar.activation(out=wmt[:], in_=wmt[:], func=EXP)
        # per-partition 0.5 bias vectors for the exp(0.5*t + 0.5) ops
        half1 = singles.tile([1, 1], F32, tag="half1")
        nc.vector.memset(half1[:], 0.5)
        half64 = singles.tile([C, 1], F32, tag="half64")
        nc.vector.memset(half64[:], 0.5)

        # ---- single big-transfer stream on the sync HWDGE ring, in exact
        # PE consumption order.
        xt_t = singles.tile([128, KO_G], BF16, tag="xt")
        nc.sync.dma_start(out=xt_t[:], in_=xt)
        bab_t = singles.tile([1, 4 * HS], F32R, tag="bab")
        nc.sync.dma_start(out=bab_t[:], in_=bab)

        # ---- PSUM tiles -------------------------------------------------
        pg_ig = psum.tile([1, 2 * HS], F32, tag="pg_ig")  # [pre_i | 2*pre_g]
        pg_o = psum.tile([1, HS], F32, tag="pg_o")        # pre_o
        pwi = psum.tile([1, HS], F32, tag="pwi")          # alpha_wi row
        pal = psum.tile([C, HS], F32, tag="pal")          # alpha pre-activation
        ps = psum.tile([1, 2 * HS], F32, tag="ps")        # [sum_ew | sum_mg]
        pdum = psum.tile([1, 2 * HS], F32, tag="pdum")    # warm scratch

        # ---- PE clock ramp: the HAM p-state ladder (0.65 -> 1.2 -> 2.4 GHz)
        # only reaches max after ~3us of GAPLESS PE activity, and any stall
        # resets it (v2 ran the whole stream at 1.2 GHz off ~300ns just-in-
        # time chunk waits).  Data-independent dummy matmuls bridge the
        # preamble -> first-weight-chunk window with zero gaps; the real
        # stream then starts with ~2 chunks already resident and the DMA
        # pulling further ahead, so the ramp completes at ~3us undisturbed.
        for _ in range(7):
            nc.tensor.matmul(pdum[:, 0:HS], lhsT=warm_t[:, 0:1], rhs=warm_t[:],
                             start=True, stop=True)

        # gates bias rows via K=1 rank-1 matmuls (open the PSUM groups)
        nc.tensor.matmul(pg_ig[:], lhsT=ones_b[0:1, 0:1], rhs=bab_t[:, 0:512],
                         start=True, stop=False)
        nc.tensor.matmul(pg_o[:], lhsT=ones_b[0:1, 0:1], rhs=bab_t[:, 512:768],
                         start=True, stop=False)

        # ---- [i|g] gates stream: ramp-up chunk sizes so the PE gets work
        # as soon as bytes land; DMA runs ahead of the PE after that.
        ig_sizes = [1, 2, 4, 8, 8, 9]
        k0 = 0
        for sz in ig_sizes:
            t = ig_pool.tile([128, 9, 2 * HS], FP8, tag="wig")
            nc.sync.dma_start(out=t[:, 0:sz, :], in_=wig[:, k0 : k0 + sz, :])
            for km in range(sz):
                kk = k0 + km
                nc.tensor.matmul(
                    pg_ig[:],
                    lhsT=xt_t[:, kk : kk + 1],
                    rhs=t[:, km, :],
                    start=False,
                    stop=(kk == KO_G - 1),
                )
            k0 += sz

        # ---- alpha weights / c_input^T, then o weights ------------------
        wa_t = singles.tile([128, 2 * KO_A, HS], FP8, tag="wa")
        ct_t = singles.tile([128, KO_A, C], BF16, tag="ct")
        nc.sync.dma_start(out=wa_t[:], in_=wa)
        nc.sync.dma_start(out=ct_t[:], in_=ct)
        cs_t = singles.tile([C, HS], F32R, tag="cs")
        nc.sync.dma_start(out=cs_t[:], in_=cs)

        o_sizes = [16, 16]
        o_tiles = []
        k0 = 0
        for sz in o_sizes:
            t = o_pool.tile([128, 16, HS], FP8, tag="wo")
            nc.sync.dma_start(out=t[:, 0:sz, :], in_=wo[:, k0 : k0 + sz, :])
            o_tiles.append((t, k0, sz))
            k0 += sz

        # ---- ig tail part 1 (ACT/DVE; overlaps alpha matmuls below) -----
        # tio = tanh(pre/(2S)) -> [t_i | t_g] with t_g = tanh(pre_g) exactly
        # (g weights are host-doubled).
        tio = singles.tile([1, 2 * HS], F32, tag="tio")
        nc.scalar.activation(out=tio[:], in_=pg_ig[:], func=TANH, scale=INV2S)
        # ewmg = [ew64 | mg64]: ew64 = exp(sig_i) = exp(0.5*t_i + 0.5);
        # mg64 = g * ew64
        ewmg = singles.tile([1, 2 * HS], F32, tag="ewmg")
        nc.scalar.activation(out=ewmg[:, 0:HS], in_=tio[:, 0:HS], func=EXP,
                             scale=0.5, bias=half1[:])
        nc.vector.tensor_tensor(out=ewmg[:, HS : 2 * HS], in0=tio[:, HS : 2 * HS],
                                in1=ewmg[:, 0:HS], op=MUL)

        # ---- alpha matmuls ----------------------------------------------
        # alpha_wi = input_ @ aW_ih  (input_ = xt cols 16..31)
        for ko in range(KO_A):
            nc.tensor.matmul(
                pwi[:],
                lhsT=xt_t[:, KO_A + ko : KO_A + ko + 1],
                rhs=wa_t[:, ko, :],
                start=(ko == 0),
                stop=(ko == KO_A - 1),
            )
        # alpha pre = c_input @ aW_hh
        for ko in range(KO_A):
            nc.tensor.matmul(
                pal[:],
                lhsT=ct_t[:, ko, :],
                rhs=wa_t[:, KO_A + ko, :],
                start=(ko == 0),
                stop=False,
            )
        # wi row (+ alpha_bias) to SBUF, broadcast-added into pal via a K=1
        # rank-1 matmul with a ones column.
        wi_t = singles.tile([1, HS], F32R, tag="wi")
        nc.vector.tensor_tensor(out=wi_t[:], in0=pwi[:], in1=bab_t[:, 768:1024],
                                op=ADD)
        nc.tensor.matmul(pal[:], lhsT=ones_b[0:1, 0:C], rhs=wi_t[:],
                         start=False, stop=True)

        # ---- alpha tail (ACT/DVE; overlaps the o-gate stream) -----------
        tal = singles.tile([C, HS], F32, tag="tal")
        nc.scalar.activation(out=tal[:], in_=pal[:], func=TANH, scale=INV2S)
        ew_al = singles.tile([C, HS], F32R, tag="ew_al")
        nc.scalar.activation(out=ew_al[:], in_=tal[:], func=EXP,
                             scale=0.5, bias=half64[:])
        mg_al = singles.tile([C, HS], F32R, tag="mg_al")
        nc.vector.tensor_tensor(out=mg_al[:], in0=cs_t[:], in1=ew_al[:], op=MUL)

        # ---- o-gate matmuls, reduction interleaved ----------------------
        for t, k0, sz in o_tiles:
            for km in range(sz):
                kk = k0 + km
                nc.tensor.matmul(
                    pg_o[:],
                    lhsT=xt_t[:, kk : kk + 1],
                    rhs=t[:, km, :],
                    start=False,
                    stop=(kk == KO_G - 1),
                )
                if kk == KO_A + 7:
                    # (C)-axis exp-normalize reductions over the 64 alpha
                    # rows (the i/g row joins via a DVE add below, off the
                    # PE).  Emitted ~8 chunks into the o stream: late enough
                    # that the alpha ACT/DVE chain is done (no PE stall),
                    # early enough that the c1 chain overlaps the o stream.
                    nc.tensor.matmul(ps[:, 0:HS], lhsT=ones_r[:], rhs=ew_al[:],
                                     start=True, stop=True)
                    nc.tensor.matmul(ps[:, HS : 2 * HS], lhsT=ones_r[:],
                                     rhs=mg_al[:], start=True, stop=True)

        # ---- c1 chain (overlaps the tail of the o stream) ---------------
        st = singles.tile([1, 2 * HS], F32, tag="st")
        nc.vector.tensor_tensor(out=st[:], in0=ps[:], in1=ewmg[:], op=ADD)
        r_t = singles.tile([1, HS], F32, tag="r")
        nc.vector.reciprocal_approx_fast(out=r_t[:], in_=st[:, 0:HS])
        hc_t = singles.tile([1, 2 * HS], F32, tag="hc")
        nc.vector.tensor_tensor(out=hc_t[:, 0:HS], in0=st[:, HS : 2 * HS],
                                in1=r_t[:], op=MUL)
        # c1 half goes out immediately, overlapping the rest
        nc.sync.dma_start(out=hc[:, 0:HS], in_=hc_t[:, 0:HS])
        t4 = singles.tile([1, HS], F32, tag="t4")
        nc.scalar.activation(out=t4[:], in_=hc_t[:, 0:HS], func=TANH)

        # ---- o tail: sig_o = 0.5 + 0.5*tanh(pre_o/(2S)); h1 = sig_o*t4 --
        to_t = singles.tile([1, HS], F32, tag="to")
        nc.scalar.activation(out=to_t[:], in_=pg_o[:], func=TANH, scale=INV2S)
        nc.vector.tensor_scalar(out=to_t[:], in0=to_t[:], scalar1=0.5,
                                scalar2=0.5, op0=MUL, op1=ADD)
        nc.vector.tensor_tensor(out=hc_t[:, HS : 2 * HS], in0=to_t[:],
                                in1=t4[:], op=MUL)
        nc.sync.dma_start(out=hc[:, HS : 2 * HS], in_=hc_t[:, HS : 2 * HS])


def _shard_inputs(input_, c_input, h0, c0, weight_ih, weight_hh,
                  alpha_weight_ih, alpha_weight_hh, bias, alpha_bias):
    """Host-side scatter: column-shard the weights over the hidden dim.

    Weights are scaled by 64 (g block by 128), clipped to the e3m4 range and
    cast to fp8 once; per-core shards are then cheap slices.
    """
    import ml_dtypes
    f32 = np.float32
    bf16 = ml_dtypes.bfloat16
    e3m4 = ml_dtypes.float8_e3m4

    x_comb = np.concatenate([h0[0], input_[0]]).astype(f32)          # (4096,)
    xt = np.ascontiguousarray(x_comb.reshape(KO_G, 128).T).astype(bf16)
    # c_input.T tiled to [ki=128, ko=16, C]
    ct = np.ascontiguousarray(
        c_input.T.reshape(KO_A, 128, C)).transpose(1, 0, 2)
    ct = np.ascontiguousarray(ct).astype(bf16)

    def q8(x):
        return np.clip(x * f32(SCALE), -15.5, 15.5).astype(e3m4)

    # gates weights: stack [W_hh; W_ih]; quantize i/o at x64, g at x128.
    wg_full = np.concatenate([weight_hh, weight_ih], axis=0).astype(f32)
    wq_i = q8(wg_full[:, 0:H])
    wq_o = q8(wg_full[:, H : 2 * H])
    wq_g = q8(wg_full[:, 2 * H : 3 * H] * f32(2.0))
    del wg_full

    def tile_k(w):  # [4096, n] -> [128, 32, n]
        n = w.shape[1]
        return np.ascontiguousarray(w.reshape(KO_G, 128, n).transpose(1, 0, 2))

    wa_full = np.concatenate([alpha_weight_ih, alpha_weight_hh], axis=0)
    wa_q = q8(wa_full.astype(f32))
    del wa_full

    bias = np.asarray(bias, f32)
    alpha_bias = np.asarray(alpha_bias, f32)
    c_input = np.asarray(c_input, f32)

    in_maps = []
    for k in range(NCORES):
        cols = np.s_[k * HS : (k + 1) * HS]
        wig = tile_k(np.concatenate([wq_i[:, cols], wq_g[:, cols]], axis=1))
        wo = tile_k(wq_o[:, cols])
        wa = np.ascontiguousarray(
            wa_q[:, cols].reshape(2 * KO_A, 128, HS).transpose(1, 0, 2))
        bab = np.concatenate(
            [bias[0 * H + k * HS : 0 * H + (k + 1) * HS] * f32(SCALE),
             bias[2 * H + k * HS : 2 * H + (k + 1) * HS] * f32(2 * SCALE),
             bias[1 * H + k * HS : 1 * H + (k + 1) * HS] * f32(SCALE),
             alpha_bias[cols] * f32(SCALE)])[None, :].astype(f32)
        in_maps.append({
            "wig": wig,
            "wo": wo,
            "wa": wa,
            "bab": bab,
            "cs": np.ascontiguousarray(c_input[:, cols]),
            "xt": xt,
            "ct": ct,
        })
    return in_maps


def _run(inputs, trace=False):
    global _nc_cache
    if _nc_cache is None:
        _nc_cache = _build_nc()
    nc = _nc_cache
    in_maps = _shard_inputs(**inputs)
    res = run_bass_kernel_spmd(nc, in_maps, core_ids=list(range(NCORES)), trace=trace)
    h1 = np.concatenate(
        [res.results[k]["hc"][:, HS : 2 * HS] for k in range(NCORES)], axis=1)
    c1 = np.concatenate(
        [res.results[k]["hc"][:, 0:HS] for k in range(NCORES)], axis=1)
    return (h1.astype(np.float32), c1.astype(np.float32)), res


def kernel(input_, c_input, h0, c0, weight_ih, weight_hh,
           alpha_weight_ih, alpha_weight_hh, bias, alpha_bias):
    inputs = dict(
        input_=np.asarray(input_, np.float32),
        c_input=np.asarray(c_input, np.float32),
        h0=np.asarray(h0, np.float32),
        c0=np.asarray(c0, np.float32),
        weight_ih=np.asarray(weight_ih, np.float32),
        weight_hh=np.asarray(weight_hh, np.float32),
        alpha_weight_ih=np.asarray(alpha_weight_ih, np.float32),
        alpha_weight_hh=np.asarray(alpha_weight_hh, np.float32),
        bias=np.asarray(bias, np.float32),
        alpha_bias=np.asarray(alpha_bias, np.float32),
    )
    out, _ = _run(inputs)
    return out


# revision 14
# speedup vs baseline: 2.8840x; 2.7458x over previous
# Self-contained Trainium2 Bass kernel for nn_MultiInputLSTMCell.
#
# Reference computation (all fp32):
#   pre   = h0 @ W_hh + bias + input_ @ W_ih          # (1, 3H)
#   i, o  = sigmoid(pre[:, :H]), sigmoid(pre[:, H:2H])
#   g     = tanh(pre[:, 2H:])
#   awi   = input_ @ aW_ih + a_bias                   # (1, H)
#   awh   = c_input @ aW_hh                           # (C, H)
#   alpha = sigmoid(awi + awh)                        # (C, H)
#   w     = exp([i; alpha]); w /= w.sum(0)            # (C+1, H)
#   c1    = (([g; c_input]) * w).sum(0)               # (1, H)
#   h1    = o * tanh(c1)
#
# Strategy: tensor-parallel over the hidden (output-column) dim across 8
# cores (HS = 256 columns each); everything after the matmuls is local to a
# shard, so no collectives.
#
# Key design points (from trace analysis; baseline bf16 kernel = 43.7us):
#  * Weights host-quantized to fp8 E3M4 (4 mantissa bits), x64 pre-scale
#    (x128 for the g block so one tanh(x/(2*64)) serves sigmoid(i,o) and
#    tanh(g) alike; aW_hh at x32 against a x2 E3M4 c_input^T so products
#    stay x64).  Measured end-to-end err ~8.7e-3 vs the 2e-2 gate; E4M3
#    (3 mantissa bits) measures 2.4e-2 and fails.  fp8 halves HBM bytes vs
#    bf16 (~4.4 MB/core) and float8e3 streams the PE at full 1 col/cycle.
#  * All sigmoids via 0.5+0.5*tanh(x/2): tanh and exp share one ACT table
#    set ("exp_and_others"), so no mid-kernel table reloads and the tail is
#    ~half the ops of the exp+reciprocal formulation.
#  * The PE p-state ladder (0.65 -> 1.2 -> 2.4 GHz) needs ~3us of GAPLESS
#    activity to reach max and resets on stalls; 10 dummy matmuls on memset
#    data bridge engine-init -> first-weight-data so the ramp completes
#    undisturbed (without this the whole stream runs at 1.2 GHz).
#  * DMA completion semaphores fire ~1.2us after the data lands and each
#    dma_start costs ~0.65us of SP sequencer time, so: few, large chunk
#    DMAs into pre-allocated whole tiles (no pool-recycling backpressure),
#    issued in exact PE consumption order; small side tensors (c_input^T,
#    merge values) ride the second (scalar) HWDGE ring in parallel.
#  * PE order wi -> alpha -> [i|g] gates -> o gates: the serial normalize/
#    c1/tanh(c1) chain overlaps the o-gate stream, and the [i;alpha] rows
#    join the exp-normalize as a single K=65 ones-matmul (bias rows enter
#    as rank-1 matmuls at the accumulation-group edges, off the hot path).
#  * Fixed costs measured on this setup: ~7us framework preamble + ~3.6us
#    final-DMA+teardown; a 1-DMA kernel measures 13.5us total.

import numpy as np

import concourse.bass as bass
import concourse.tile as tile
from concourse import bacc, mybir
from concourse.bass_utils import run_bass_kernel_spmd

NCORES = 8
H = 2048          # hidden size
IN = 2048         # input size
C = 64            # number of skip-word cell states
HS = H // NCORES  # hidden shard per core = 256
KG = IN + H       # gates contraction dim = 4096
KO_G = KG // 128  # 32 k-chunks for gates
KO_A = IN // 128  # 16 k-chunks per alpha matmul
SCALE = 64.0      # fp8 weight pre-scale
F32 = mybir.dt.float32
F32R = mybir.dt.float32r
BF16 = mybir.dt.bfloat16
FP8 = mybir.dt.float8e3

_nc_cache = None


def _build_nc():
    """Build the single-core Bass program (same program runs on all 8 cores)."""
    nc = bacc.Bacc(
        "TRN2",
        target_bir_lowering=False,
        debug=False,
        enable_asserts=False,
        name="multi_input_lstm_cell",
    )

    # DRAM I/O (per-core shards; shapes identical on every core).  Weights
    # host-pre-tiled to [ki=128, ko, n]: chunk DMAs read one contiguous
    # segment per partition.
    # wig columns: [i-gate shard (256) | 2*g-gate shard (256)], e3m4 * 64
    wig = nc.dram_tensor("wig", [128, KO_G, 2 * HS], FP8, kind="ExternalInput").ap()
    wo = nc.dram_tensor("wo", [128, KO_G, HS], FP8, kind="ExternalInput").ap()
    # wa rows 0..2047 = 64*aW_ih shard, rows 2048..4095 = 32*aW_hh shard
    wa = nc.dram_tensor("wa", [128, 2 * KO_A, HS], FP8, kind="ExternalInput").ap()
    # bab = [64*b_i | 128*b_g | 64*b_o | 64*ab]
    bab = nc.dram_tensor("bab", [1, 4 * HS], F32R, kind="ExternalInput").ap()
    cs = nc.dram_tensor("cs", [C, HS], F32R, kind="ExternalInput").ap()
    xt = nc.dram_tensor("xt", [128, KO_G], BF16, kind="ExternalInput").ap()
    ct = nc.dram_tensor("ct", [128, KO_A, C], FP8, kind="ExternalInput").ap()
    # hc[0, 0:256] = c1 shard, hc[0, 256:512] = h1 shard
    hc = nc.dram_tensor("hc", [1, 2 * HS], F32, kind="ExternalOutput").ap()

    with tile.TileContext(nc) as tc:
        _emit(tc, wig, wo, wa, bab, cs, xt, ct, hc)

    nc.compile()
    return nc


def _emit(tc, wig, wo, wa, bab, cs, xt, ct, hc):
    from contextlib import ExitStack

    nc = tc.nc
    TANH = mybir.ActivationFunctionType.Tanh
    EXP = mybir.ActivationFunctionType.Exp
    MUL = mybir.AluOpType.mult
    ADD = mybir.AluOpType.add
    INV2S = 1.0 / (2.0 * SCALE)

    with ExitStack() as ctx:
        singles = ctx.enter_context(tc.tile_pool(name="singles", bufs=1))
        psum = ctx.enter_context(tc.tile_pool(name="psum", bufs=1, space="PSUM"))

        # ---- memset-sourced tiles (no DMA dependency) --------------------
        warm_t = singles.tile([128, HS], BF16, tag="warm")
        nc.vector.memset(warm_t[:], 0.0)
        ones_b_f = singles.tile([1, C], F32, tag="ones_b")
        nc.vector.memset(ones_b_f[:], 1.0)
        ones_b = ones_b_f[:].bitcast(F32R)
        ones_r_f = singles.tile([C + 1, 1], F32, tag="ones_r")
        nc.vector.memset(ones_r_f[:], 1.0)
        ones_r = ones_r_f[:].bitcast(F32R)
        wmt = singles.tile([1, 1], F32, tag="wmt")
        nc.vector.memset(wmt[:], 0.0)
        nc.scalar.activation(out=wmt[:], in_=wmt[:], func=EXP)  # table prewarm
        half1 = singles.tile([1, 1], F32, tag="half1")
        nc.vector.memset(half1[:], 0.5)
        half64 = singles.tile([C, 1], F32, tag="half64")
        nc.vector.memset(half64[:], 0.5)

        # ---- weight/side tiles (whole tensors; DMAs write disjoint slices,
        # so no pool recycling and no backpressure on the issue stream) ----
        xt_t = singles.tile([128, KO_G], BF16, tag="xt")
        wa_t = singles.tile([128, 2 * KO_A, HS], FP8, tag="wa")
        bab_t = singles.tile([1, 4 * HS], F32R, tag="bab")
        wig_t = singles.tile([128, KO_G, 2 * HS], FP8, tag="wig")
        wo_t = singles.tile([128, KO_G, HS], FP8, tag="wo")
        ct_t = singles.tile([128, KO_A, C], FP8, tag="ct")
        cs_t = singles.tile([C, HS], F32R, tag="cs")

        # sync-ring stream, in PE consumption order
        nc.sync.dma_start(out=xt_t[:], in_=xt)
        for a in range(0, 2 * KO_A, 8):
            if a == 8:
                nc.sync.dma_start(out=bab_t[:], in_=bab)
            nc.sync.dma_start(out=wa_t[:, a : a + 8, :], in_=wa[:, a : a + 8, :])
        for a in range(0, KO_G, 8):
            nc.sync.dma_start(out=wig_t[:, a : a + 8, :], in_=wig[:, a : a + 8, :])
        for a in range(0, KO_G, 16):
            nc.sync.dma_start(out=wo_t[:, a : a + 16, :], in_=wo[:, a : a + 16, :])
        # side tensors on the scalar HWDGE ring (runs in parallel; these are
        # small and consumed mid-kernel)
        nc.scalar.dma_start(out=ct_t[:], in_=ct)
        nc.scalar.dma_start(out=cs_t[:], in_=cs)

        # ---- PSUM tiles --------------------------------------------------
        pg_ig = psum.tile([1, 2 * HS], F32, tag="pg_ig")  # [pre_i | 2*pre_g]
        pg_o = psum.tile([1, HS], F32, tag="pg_o")        # pre_o
        pwi = psum.tile([1, HS], F32, tag="pwi")          # alpha_wi row
        pal = psum.tile([C, HS], F32, tag="pal")          # alpha pre-activation
        ps = psum.tile([1, 2 * HS], F32, tag="ps")        # [sum_ew | sum_mg]
        pdum = psum.tile([1, HS], F32, tag="pdum")        # warm scratch

        # ---- PE ramp dummies (gap-free by construction) ------------------
        for _ in range(10):
            nc.tensor.matmul(pdum[:], lhsT=warm_t[:, 0:1], rhs=warm_t[:],
                             start=True, stop=True)

        # ---- alpha_wi = input_ @ aW_ih  (input_ = xt cols 16..31) --------
        for ko in range(KO_A):
            nc.tensor.matmul(
                pwi[:],
                lhsT=xt_t[:, KO_A + ko : KO_A + ko + 1],
                rhs=wa_t[:, ko, :],
                start=(ko == 0),
                stop=(ko == KO_A - 1),
            )
        # ---- alpha pre = c_input @ aW_hh ---------------------------------
        for ko in range(KO_A):
            nc.tensor.matmul(
                pal[:],
                lhsT=ct_t[:, ko, :],
                rhs=wa_t[:, KO_A + ko, :],
                start=(ko == 0),
                stop=False,
            )
        # wi row (+ alpha_bias) to SBUF, broadcast-added into pal via a K=1
        # ones matmul (closes the pal group).
        wi_t = singles.tile([1, HS], F32R, tag="wi")
        nc.vector.tensor_tensor(out=wi_t[:], in0=pwi[:], in1=bab_t[:, 768:1024],
                                op=ADD)
        nc.tensor.matmul(pal[:], lhsT=ones_b[0:1, 0:C], rhs=wi_t[:],
                         start=False, stop=True)

        # ---- alpha tail (ACT/DVE; overlaps the ig stream) ----------------
        # rows 0..63 of the K=65 exp-normalize operands
        tal = singles.tile([C, HS], F32, tag="tal")
        nc.scalar.activation(out=tal[:], in_=pal[:], func=TANH, scale=INV2S)
        ew_t = singles.tile([C + 1, HS], F32R, tag="ew")
        mg_t = singles.tile([C + 1, HS], F32R, tag="mg")
        nc.scalar.activation(out=ew_t[0:C, :], in_=tal[:], func=EXP,
                             scale=0.5, bias=half64[:])
        nc.vector.tensor_tensor(out=mg_t[0:C, :], in0=cs_t[:], in1=ew_t[0:C, :],
                                op=MUL)

        # ---- [i|g] gates stream ------------------------------------------
        for kk in range(KO_G):
            nc.tensor.matmul(
                pg_ig[:],
                lhsT=xt_t[:, kk : kk + 1],
                rhs=wig_t[:, kk, :],
                start=(kk == 0),
                stop=False,
            )
        # gates bias joins as a K=1 rank-1 matmul closing the group (keeps
        # the first gates matmul free of the bias-DMA dependency)
        nc.tensor.matmul(pg_ig[:], lhsT=ones_b[0:1, 0:1], rhs=bab_t[:, 0:512],
                         start=False, stop=True)

        # ---- ig tail: row 64 of the normalize operands -------------------
        # tio = tanh(pre/(2S)) = [t_i | tanh(pre_g)]
        tio = singles.tile([1, 2 * HS], F32, tag="tio")
        nc.scalar.activation(out=tio[:], in_=pg_ig[:], func=TANH, scale=INV2S)
        # ew[64] = exp(sig_i) = exp(0.5*t_i + 0.5); mg[64] = g * ew[64]
        # (two steps: DVE tensor_tensor requires both SBUF inputs to share a
        # base partition, so stage g onto partition 64 first)
        nc.scalar.activation(out=ew_t[C : C + 1, :], in_=tio[:, 0:HS], func=EXP,
                             scale=0.5, bias=half1[:])
        nc.vector.tensor_scalar(out=mg_t[C : C + 1, :], in0=tio[:, HS : 2 * HS],
                                scalar1=1.0, scalar2=None, op0=MUL)
        nc.vector.tensor_tensor(out=mg_t[C : C + 1, :], in0=mg_t[C : C + 1, :],
                                in1=ew_t[C : C + 1, :], op=MUL)

        # ---- o-gate stream with the K=65 reductions interleaved ----------
        nc.tensor.matmul(pg_o[:], lhsT=ones_b[0:1, 0:1], rhs=bab_t[:, 512:768],
                         start=True, stop=False)
        for kk in range(KO_G):
            nc.tensor.matmul(
                pg_o[:],
                lhsT=xt_t[:, kk : kk + 1],
                rhs=wo_t[:, kk, :],
                start=False,
                stop=(kk == KO_G - 1),
            )
            if kk == 15:
                nc.tensor.matmul(ps[:, 0:HS], lhsT=ones_r[:], rhs=ew_t[:],
                                 start=True, stop=True)
                nc.tensor.matmul(ps[:, HS : 2 * HS], lhsT=ones_r[:], rhs=mg_t[:],
                                 start=True, stop=True)

        # ---- c1 = ps1 / ps0 ; overlaps the o-stream tail -----------------
        r_t = singles.tile([1, HS], F32, tag="r")
        nc.vector.reciprocal_approx_fast(out=r_t[:], in_=ps[:, 0:HS])
        hc_t = singles.tile([1, 2 * HS], F32, tag="hc")
        nc.vector.tensor_tensor(out=hc_t[:, 0:HS], in0=ps[:, HS : 2 * HS],
                                in1=r_t[:], op=MUL)
        nc.sync.dma_start(out=hc[:, 0:HS], in_=hc_t[:, 0:HS])
        t4 = singles.tile([1, HS], F32, tag="t4")
        nc.scalar.activation(out=t4[:], in_=hc_t[:, 0:HS], func=TANH)

        # ---- o tail: h1 = (0.5 + 0.5*tanh(pre_o/(2S))) * tanh(c1) --------
        to_t = singles.tile([1, HS], F32, tag="to")
        nc.scalar.activation(out=to_t[:], in_=pg_o[:], func=TANH, scale=INV2S)
        nc.vector.tensor_scalar(out=to_t[:], in0=to_t[:], scalar1=0.5,
                                scalar2=0.5, op0=MUL, op1=ADD)
        nc.vector.tensor_tensor(out=hc_t[:, HS : 2 * HS], in0=to_t[:],
                                in1=t4[:], op=MUL)
        nc.sync.dma_start(out=hc[:, HS : 2 * HS], in_=hc_t[:, HS : 2 * HS])


def _shard_inputs(input_, c_input, h0, c0, weight_ih, weight_hh,
                  alpha_weight_ih, alpha_weight_hh, bias, alpha_bias):
    """Host-side scatter: column-shard the weights over the hidden dim.

    Weights scaled (x64; g block x128; aW_hh x32 against x2 c_input^T),
    clipped to the e3m4 range and cast to fp8 once; per-core shards are
    cheap slices.
    """
    import ml_dtypes
    f32 = np.float32
    bf16 = ml_dtypes.bfloat16
    e3m4 = ml_dtypes.float8_e3m4

    x_comb = np.concatenate([h0[0], input_[0]]).astype(f32)          # (4096,)
    xt = np.ascontiguousarray(x_comb.reshape(KO_G, 128).T).astype(bf16)

    def q8(x, sc):
        return np.clip(np.asarray(x, f32) * f32(sc), -15.5, 15.5).astype(e3m4)

    # c_input.T tiled to [ki=128, ko=16, C], e3m4 at x2
    ct = np.ascontiguousarray(
        np.ascontiguousarray(c_input.T.reshape(KO_A, 128, C)).transpose(1, 0, 2))
    ct = q8(ct, 2.0)

    # gates weights: stack [W_hh; W_ih]; i/o at x64, g at x128.
    wg_full = np.concatenate([weight_hh, weight_ih], axis=0).astype(f32)
    wq_i = q8(wg_full[:, 0:H], SCALE)
    wq_o = q8(wg_full[:, H : 2 * H], SCALE)
    wq_g = q8(wg_full[:, 2 * H : 3 * H], 2 * SCALE)
    del wg_full

    def tile_k(w):  # [4096, n] -> [128, 32, n]
        n = w.shape[1]
        return np.ascontiguousarray(w.reshape(KO_G, 128, n).transpose(1, 0, 2))

    wa_q = np.concatenate(
        [q8(alpha_weight_ih, SCALE), q8(alpha_weight_hh, SCALE / 2)], axis=0)

    bias = np.asarray(bias, f32)
    alpha_bias = np.asarray(alpha_bias, f32)
    c_input = np.asarray(c_input, f32)

    in_maps = []
    for k in range(NCORES):
        cols = np.s_[k * HS : (k + 1) * HS]
        wig = tile_k(np.concatenate([wq_i[:, cols], wq_g[:, cols]], axis=1))
        wo = tile_k(wq_o[:, cols])
        wa = np.ascontiguousarray(
            wa_q[:, cols].reshape(2 * KO_A, 128, HS).transpose(1, 0, 2))
        bab = np.concatenate(
            [bias[0 * H + k * HS : 0 * H + (k + 1) * HS] * f32(SCALE),
             bias[2 * H + k * HS : 2 * H + (k + 1) * HS] * f32(2 * SCALE),
             bias[1 * H + k * HS : 1 * H + (k + 1) * HS] * f32(SCALE),
             alpha_bias[cols] * f32(SCALE)])[None, :].astype(f32)
        in_maps.append({
            "wig": wig,
            "wo": wo,
            "wa": wa,
            "bab": bab,
            "cs": np.ascontiguousarray(c_input[:, cols]),
            "xt": xt,
            "ct": ct,
        })
    return in_maps


def _run(inputs, trace=False):
    global _nc_cache
    if _nc_cache is None:
        _nc_cache = _build_nc()
    nc = _nc_cache
    in_maps = _shard_inputs(**inputs)
    res = run_bass_kernel_spmd(nc, in_maps, core_ids=list(range(NCORES)), trace=trace)
    h1 = np.concatenate(
        [res.results[k]["hc"][:, HS : 2 * HS] for k in range(NCORES)], axis=1)
    c1 = np.concatenate(
        [res.results[k]["hc"][:, 0:HS] for k in range(NCORES)], axis=1)
    return (h1.astype(np.float32), c1.astype(np.float32)), res


def kernel(input_, c_input, h0, c0, weight_ih, weight_hh,
           alpha_weight_ih, alpha_weight_hh, bias, alpha_bias):
    inputs = dict(
        input_=np.asarray(input_, np.float32),
        c_input=np.asarray(c_input, np.float32),
        h0=np.asarray(h0, np.float32),
        c0=np.asarray(c0, np.float32),
        weight_ih=np.asarray(weight_ih, np.float32),
        weight_hh=np.asarray(weight_hh, np.float32),
        alpha_weight_ih=np.asarray(alpha_weight_ih, np.float32),
        alpha_weight_hh=np.asarray(alpha_weight_hh, np.float32),
        bias=np.asarray(bias, np.float32),
        alpha_bias=np.asarray(alpha_bias, np.float32),
    )
    out, _ = _run(inputs)
    return out
